# revision 6
# baseline (speedup 1.0000x reference)
"""DenseCRF mean-field kernel for Trainium2 (8 NeuronCores, data parallel).

Math per sample (B=8 samples -> 1 per core):
    Q0 = softmax(unary, axis=class)
    repeat 5x:  Q <- softmax(Q - compat @ ((pos_w+bi_w) * gauss7(Q)), axis=class)
(`image` is unused by the reference math.)

The 7x7 gaussian blur is separable with reflect padding, expressed as two
banded matrix multiplies on the TensorEngine:
    pass1: T1[w, h'] = sum_h Q[h, w] * AT[h, h']              (H-conv, transposed out)
    pass2: L[h',w'] = Q[h',w'] + sum_w T1[w, h'] * (-s*AT)[w, w']  (W-conv + identity)
where AT = A.T, A the [n,n] reflect conv matrix of g, s = pos_w + bi_w
(times compat diagonal). Both passes keep the data operand stationary so the
result returns to natural [h, w] orientation; the identity matmul goes first
with start=True (clears the PSUM bank) and the overlapping band windows
accumulate per-element via PSUM has_written semantics.

Per core the sample stays resident in SBUF as fp16 between iterations; HBM
traffic is only the initial unary load and final Q store (2 x 21 MB).
"""

from contextlib import ExitStack

import numpy as np

import concourse.bacc as bacc
import concourse.tile as tile
from concourse import mybir
from concourse.bass_utils import run_bass_kernel_spmd

F32 = mybir.dt.float32
F16 = mybir.dt.float16

B, C, H, W = 8, 21, 512, 512
KSIZE, SIGMA = 7, 2.0
NUM_ITERATIONS = 5
PB = 128                       # partition block
BANDW = PB + 2 * (KSIZE // 2)  # max band window width (134)


def _gauss1d():
    coords = np.arange(KSIZE, dtype=np.float64) - KSIZE // 2
    g = np.exp(-(coords ** 2) / (2.0 * SIGMA ** 2))
    return g / g.sum()


def _conv_matrix(n, g):
    r = len(g) // 2
    A = np.zeros((n, n), np.float64)
    for i in range(n):
        for t in range(len(g)):
            j = i + t - r
            if j < 0:
                j = -j
            if j >= n:
                j = 2 * n - 2 - j
            A[i, j] += g[t]
    return A  # filt = A @ x  (reflect boundary)


def _windows(n):
    r = KSIZE // 2
    return [(max(0, PB * i - r), min(n, PB * i + PB + r)) for i in range(n // PB)]


def build_program(c=C, hb=H // PB, w=W, iters=NUM_ITERATIONS, n_cores=8,
                  b2_per_class=False, offdiag=None):
    """Build the per-core Bass program.

    offdiag: None for (scaled-)identity compat, else the full [c,c] compat
    matrix -> generic (slow) class-mix path with DRAM-resident E.
    """
    h = hb * PB
    wb = w // PB
    wins_h = _windows(h)
    wins_w = _windows(w)
    n_b2 = c if b2_per_class else 1
    generic = offdiag is not None

    nc = bacc.Bacc("TRN2", target_bir_lowering=False, debug=False,
                   num_devices=n_cores)
    U = nc.dram_tensor("unary", [c, h, w], F32, kind="ExternalInput")
    BD1 = nc.dram_tensor("band1", [hb, PB, h], F16, kind="ExternalInput")
    BD2 = nc.dram_tensor("band2", [n_b2, wb, PB, BANDW], F16, kind="ExternalInput")
    IDN = nc.dram_tensor("ident", [PB, PB], F16, kind="ExternalInput")
    OUT = nc.dram_tensor("out", [c, h, w], F32, kind="ExternalOutput")
    EDR = nc.dram_tensor("escr", [c, h, w], F16) if generic else None

    n_grp = 3 if c >= 6 else 1
    grps = np.array_split(np.arange(c), n_grp)
    grp_of, first_in_grp = {}, {}
    for gi, g in enumerate(grps):
        for k, cc in enumerate(g):
            grp_of[int(cc)] = gi
            first_in_grp[int(cc)] = (k == 0)

    with tile.TileContext(nc) as tc, ExitStack() as ctx:
        singles = ctx.enter_context(tc.tile_pool(name="singles", bufs=1))
        t1ps_pool = ctx.enter_context(tc.tile_pool(name="t1ps", bufs=3, space="PSUM"))
        lps_pool = ctx.enter_context(tc.tile_pool(name="lps", bufs=3, space="PSUM"))
        t1sb_pool = ctx.enter_context(tc.tile_pool(name="t1sb", bufs=2))
        stage_pool = ctx.enter_context(tc.tile_pool(name="stage", bufs=4))
        sums_pool = ctx.enter_context(tc.tile_pool(name="sums", bufs=2))
        mix_pool = ctx.enter_context(tc.tile_pool(name="mix", bufs=2))

        # ---- persistent / constant SBUF ----
        qres = singles.tile([PB, c, hb, w], F16, tag="qres")
        b1 = singles.tile([PB, hb, h], F16, tag="b1")
        b2 = singles.tile([PB, n_b2, wb, BANDW], F16, tag="b2")
        ident = singles.tile([PB, PB], F16, tag="ident")
        for i in range(hb):
            nc.sync.dma_start(out=b1[:, i, :], in_=BD1[i])
        for j in range(n_b2):
            for i in range(wb):
                nc.sync.dma_start(out=b2[:, j, i, :], in_=BD2[j, i])
        nc.sync.dma_start(out=ident[:], in_=IDN[:])

        spart = {}

        def accum_E(cc, m2, e_ap):
            gi = grp_of[cc]
            if first_in_grp[cc]:
                t = sums_pool.tile([PB, w], F16, tag=f"sp_{gi}_{m2}")
                spart[(gi, m2)] = t
                nc.vector.tensor_copy(out=t[:], in_=e_ap)
            else:
                t = spart[(gi, m2)]
                nc.vector.tensor_add(out=t[:], in0=t[:], in1=e_ap)

        def emit_exp(cc, m2, src_ap):
            """exp(src) -> E storage (+ partial sums). Returns nothing."""
            if not generic:
                etgt = qres[:, cc, m2, :]
                nc.scalar.activation(out=etgt, in_=src_ap,
                                     func=mybir.ActivationFunctionType.Exp)
                accum_E(cc, m2, etgt)
            else:
                est = stage_pool.tile([PB, w], F16, tag="est")
                nc.scalar.activation(out=est[:], in_=src_ap,
                                     func=mybir.ActivationFunctionType.Exp)
                accum_E(cc, m2, est[:])
                nc.sync.dma_start(out=EDR[cc, m2 * PB:(m2 + 1) * PB, :],
                                  in_=est[:])

        def finish_round(last):
            rh = []
            for m2 in range(hb):
                s = sums_pool.tile([PB, w], F32, tag=f"s_{m2}")
                if n_grp == 1:
                    nc.vector.tensor_copy(out=s[:], in_=spart[(0, m2)][:])
                else:
                    nc.vector.tensor_add(out=s[:], in0=spart[(0, m2)][:],
                                         in1=spart[(1, m2)][:])
                    for gi in range(2, n_grp):
                        nc.vector.tensor_add(out=s[:], in0=s[:],
                                             in1=spart[(gi, m2)][:])
                r = sums_pool.tile([PB, w], F32, tag=f"r_{m2}")
                nc.vector.reciprocal_approx_fast(out=r[:], in_=s[:])
                rhm = sums_pool.tile([PB, w], F16, tag=f"rh_{m2}")
                nc.vector.tensor_copy(out=rhm[:], in_=r[:])
                rh.append(rhm)
            for cc in range(c):
                for m2 in range(hb):
                    if generic:
                        esrc = stage_pool.tile([PB, w], F16, tag="eld")
                        nc.sync.dma_start(
                            out=esrc[:], in_=EDR[cc, m2 * PB:(m2 + 1) * PB, :])
                        esrc = esrc[:]
                    else:
                        esrc = qres[:, cc, m2, :]
                    if not last:
                        nc.vector.tensor_mul(out=qres[:, cc, m2, :],
                                             in0=esrc, in1=rh[m2][:])
                    else:
                        fo = stage_pool.tile([PB, w], F32, tag="fout")
                        nc.vector.tensor_mul(out=fo[:], in0=esrc, in1=rh[m2][:])
                        nc.sync.dma_start(
                            out=OUT[cc, m2 * PB:(m2 + 1) * PB, :], in_=fo[:])
            spart.clear()

        # ---- init: Q0 = softmax(unary) ----
        for cc in range(c):
            for m2 in range(hb):
                st = stage_pool.tile([PB, w], F32, tag="uin")
                nc.sync.dma_start(out=st[:], in_=U[cc, m2 * PB:(m2 + 1) * PB, :])
                emit_exp(cc, m2, st[:])
        finish_round(last=False)

        # ---- iterations ----
        for k in range(iters):
            for cc in range(c):
                if generic:
                    msrc = mix_pool.tile([PB, hb, w], F16, tag="mix")
                    for i in range(hb):
                        nz = [j for j in range(c) if offdiag[cc, j] != 0.0]
                        if not nz:
                            nc.vector.memset(msrc[:, i, :], 0.0)
                        else:
                            j0 = nz[0]
                            nc.vector.tensor_scalar_mul(
                                out=msrc[:, i, :], in0=qres[:, j0, i, :],
                                scalar1=float(offdiag[cc, j0]))
                            for j in nz[1:]:
                                nc.vector.scalar_tensor_tensor(
                                    out=msrc[:, i, :], in0=qres[:, j, i, :],
                                    scalar=float(offdiag[cc, j]),
                                    in1=msrc[:, i, :],
                                    op0=mybir.AluOpType.mult,
                                    op1=mybir.AluOpType.add)

                    def src_ap(i, mcols):
                        return msrc[:, i, mcols]
                else:
                    def src_ap(i, mcols):
                        return qres[:, cc, i, mcols]

                # pass1: T1[w, h']
                t1sb = t1sb_pool.tile([PB, wb, h], F16, tag="t1sb")
                for m in range(wb):
                    t1ps = t1ps_pool.tile([PB, h], F32, tag="t1ps")
                    # first mm full-width: initializes the whole PSUM bank
                    # (rhs is zero outside the band), rest use band windows
                    nc.tensor.matmul(
                        t1ps[:, 0:h],
                        src_ap(0, slice(m * PB, (m + 1) * PB)),
                        b1[:, 0, :],
                        start=True, stop=(hb == 1))
                    for i in range(1, hb):
                        lo, hi = wins_h[i]
                        nc.tensor.matmul(
                            t1ps[:, lo:hi],
                            src_ap(i, slice(m * PB, (m + 1) * PB)),
                            b1[:, i, lo:hi],
                            start=False, stop=(i == hb - 1))
                    nc.scalar.copy(out=t1sb[:, m, :], in_=t1ps[:])
                # pass2: L = Q - s * blur, directly in PSUM
                b2c = b2[:, cc if n_b2 > 1 else 0]
                for m2 in range(hb):
                    lps = lps_pool.tile([PB, w], F32, tag="lps")
                    nc.tensor.matmul(lps[:, 0:w], ident[:],
                                     qres[:, cc, m2, :],
                                     start=True, stop=False)
                    for i2 in range(wb):
                        lo, hi = wins_w[i2]
                        nc.tensor.matmul(
                            lps[:, lo:hi],
                            t1sb[:, i2, m2 * PB:(m2 + 1) * PB],
                            b2c[:, i2, 0:hi - lo],
                            start=False, stop=(i2 == wb - 1))
                    emit_exp(cc, m2, lps[:])
            finish_round(last=(k == iters - 1))

    nc.compile()
    return nc


def _prep_consts(c, h, w, scale, compat):
    g = _gauss1d()
    AT_h = _conv_matrix(h, g).T
    AT_w = _conv_matrix(w, g).T
    band1 = np.zeros((h // PB, PB, h), np.float16)
    for i in range(h // PB):
        band1[i] = AT_h[i * PB:(i + 1) * PB, :].astype(np.float16)

    diag = np.diag(compat).astype(np.float64)
    is_diag = bool(np.count_nonzero(compat - np.diag(diag)) == 0)
    uniform = is_diag and bool(np.all(diag == diag[0]))

    offdiag = None
    if is_diag:
        n_b2 = 1 if uniform else c
        scales = [float(scale) * float(diag[0])] if uniform else \
                 [float(scale) * float(d) for d in diag]
    else:
        n_b2 = 1
        scales = [float(scale)]
        offdiag = compat.astype(np.float64)

    band2 = np.zeros((n_b2, w // PB, PB, BANDW), np.float16)
    for j in range(n_b2):
        for i, (lo, hi) in enumerate(_windows(w)):
            band2[j, i, :, 0:hi - lo] = (
                -scales[j] * AT_w[i * PB:(i + 1) * PB, lo:hi]).astype(np.float16)
    ident = np.eye(PB, dtype=np.float16)
    return band1, band2, ident, (n_b2 > 1), offdiag


_prog_cache = {}


def kernel(unary, image, pos_w, bi_w, compatibility):
    unary = np.asarray(unary, dtype=np.float32)
    compat = np.asarray(compatibility, dtype=np.float32)
    scale = float(np.asarray(pos_w)) + float(np.asarray(bi_w))
    b, c, h, w = unary.shape
    assert (b, c, h, w) == (B, C, H, W), (b, c, h, w)

    band1, band2, ident, per_class, offdiag = _prep_consts(c, h, w, scale, compat)
    key = (scale, compat.tobytes())
    if key not in _prog_cache:
        _prog_cache[key] = build_program(
            c=c, hb=h // PB, w=w, iters=NUM_ITERATIONS, n_cores=B,
            b2_per_class=per_class, offdiag=offdiag)
    nc = _prog_cache[key]

    in_maps = [{"unary": unary[i], "band1": band1, "band2": band2,
                "ident": ident} for i in range(B)]
    res = run_bass_kernel_spmd(nc, in_maps, list(range(B)))
    out = np.stack([res.results[i]["out"] for i in range(B)], axis=0)
    return out.astype(np.float32)


if __name__ == "__main__":
    rng = np.random.default_rng(0)
    u = rng.standard_normal((B, C, H, W), dtype=np.float32)
    img = rng.random((B, 3, H, W), dtype=np.float32)
    o = kernel(u, img, np.float32(3.0), np.float32(10.0),
               np.eye(C, dtype=np.float32))
    print(o.shape, o.dtype, float(o.sum()))
